# revision 36
# baseline (speedup 1.0000x reference)
"""CTLSTM cell fused kernel for 8 Trainium2 NeuronCores.

Strategy (data-parallel over batch, TRANSPOSED compute):
  - B=16384 rows sharded 2048/core; weights replicated.
  - Compute g.T: gates on SBUF partitions, batch on the free dim.
    Stationary operand = weight tile [K=128, 128 gates]; moving operand =
    xh [K=128, 1024 batch] bf16 (max bf16 moving free dim). Each PSUM tile
    is one gate-tile x batch-half: [128, 1024] fp32 (2 banks).
  - With gates on partitions the bias is per-partition: the ACT engine
    drains PSUM directly with out = act(psum*scale + bias[p]) in ONE op -
    no DVE bias-add drain at all. DVE only runs the elementwise chain.
  - Gate-group exec order [d, z, i, f, ib, fb, o]:
      * d first: softplus(wd) = -ln(sigmoid(-wd)). The Ln burst (one act
        table switch to natural_log and back) slots in right after the
        4 d-tiles of each half, mid-stream where ACT has slack.
      * o last: the tail after the final matmul is just sigmoid(o) +
        h = o*tanh(c) + store.
  - DMA priority: xh half-0 chunks first, weights as one ~1 MiB DMA per
    gate-group (d-q0 split into its own tile, and its k0 column-tile
    split again to 32 KiB so the first matmul's dependency closure is
    minimal); few Sync issues keep the ramp short and weight arrival
    stays ahead of the PE for the rest of the kernel.
  - Outputs h/o/dr stored bf16 (halved write traffic), c/cb fp32.
    Host transposes back and upcasts.

  - PE CLOCK WARM-UP + PACING: the tensor engine's DVFS ramps over ~3 us
    of sustained use and resets on idle gaps over ~1 us. Eight dummy
    matmuls on a memset tile run while the first DMAs stream (PE would
    idle anyway), so the real stream starts at full 2.4 GHz; two filler
    matmuls after each of the d-q0 k4/k5/k6 pairs bridge the xh07
    arrival wait so the clock never drops mid-ramp. Trace-verified:
    real-stream busy 194.8 -> 193.4 us, window 196.7 -> 195.3 us.

Measured: 213.0-216.5 us HW exec with warm-up+pacing (median 214.5,
best 213.0) vs 214.5-218.0 us without (median 215.7) - identical-code
run noise is +-1.8 us, so compare medians/trace metrics, not single
runs. Naive baseline 302.5 us. bf16 matmul roofline 191.1 us; real PE
busy 193.4 us (excess = hw-decode 2.2 ns/instr + 17 periodic ~216 ns
instruction-buffer refills every ~100 PE instructions). Head ~12 us =
7.2 fixed NEFF preamble + 3 MB ramp-critical DMA at ~370 GB/s + 0.9 us
DMA sem-prop. Tail ~6 us = sigmoid+mul+store chain (issue 625 + DGE
650 + sem-prop 900 fixed, compute-gated) + ~2.8 us teardown.
Theoretical floor with this structure ~210 us.

Tried and rejected (do not retry):
  - fp8 e4m3 DoubleRow matmul (1.44x PE measured): numerically FAILS the
    2e-2 gate - e4m3 both-operands gives 0.081 worst scale-rel, w-split
    0.062; the 3-pass hi/lo split passes (0.0027) but costs 3/1.44 = 2.1x
    bf16 time. bf16 is the floor for this accuracy.
  - 1024-col moving matmul (halve instr count): walrus ISA check rejects
    it (s3d3_mm_num_elements); 512 is the hw max despite bass not
    asserting.
  - k-wave ramp (d-group k-outer over 4 open PSUM groups + k-major
    staged xh/w0) with waves on sync: scheduler re-plans globally;
    measured 215.2-226.6 us across 4 variants - never beat this order.
  - weight DMAs on the Act HWDGE queue: Q_X starts ~2.9 us after issue
    and steals ~50% bandwidth from the sync ramp when active; anything
    the first 15 us needs must be on sync, in this exact order.
  - final o-store on Act queue: the DIRECT2D issue on the Scalar
    sequencer delays its own final sigmoids; +1.8 us.
  - walrus --enable-ldw-opt=true (merge the 448 redundant consecutive
    LDWEIGHTS from the bank-pair weight reuse): crashes codegen in
    visitInstLdweights (CoreV3GenImpl.cpp:694) - disabled for a reason.
Run-to-run noise on shared hw is larger than it looks: identical code
measured 214.5-218.0 us across 5 runs in one session.
"""

import numpy as np
import ml_dtypes

import concourse.bacc as bacc
import concourse.bass as bass
import concourse.mybir as mybir
import concourse.tile as tile
from concourse.bass_utils import run_bass_kernel_spmd

NCORES = 8
B = 16384
I = 512
H = 512
NG = 7
G = NG * H          # 3584
K2 = I + H          # 1024
P = 128
BS = B // NCORES    # 2048 batch cols per core
NH = 2              # batch halves of 1024
BN = BS // NH       # 1024
NQ = H // P         # 4 hidden quadrants (128 gate rows each)
NK = K2 // P        # 8 contraction chunks
NGT = G // P        # 28 gate tiles

BF16 = mybir.dt.bfloat16
F32 = mybir.dt.float32
AF = mybir.ActivationFunctionType
NPBF16 = ml_dtypes.bfloat16

# gate-group exec order: d, z, i, f, ib, fb, o
# reference row order is   i, f, z, o, d, ib, fb
SRC = [4, 2, 0, 1, 5, 6, 3]
GD, GZ, GI, GF, GIB, GFB, GO = range(7)

TRACE = False
LAST_RESULTS = None

_nc_cache = None


def _build():
    nc = bacc.Bacc("TRN2", target_bir_lowering=False, debug=False)

    xh_d = nc.dram_tensor("xh", [NH, NK, P, BN], BF16, kind="ExternalInput")
    w_d = nc.dram_tensor("w", [7, P, NQ * NK * P], BF16, kind="ExternalInput")
    ct_d = nc.dram_tensor("ct", [H, BS], BF16, kind="ExternalInput")
    bb_d = nc.dram_tensor("bb", [P, NGT], F32, kind="ExternalInput")

    h_d = nc.dram_tensor("h", [H, BS], BF16, kind="ExternalOutput")
    c_d = nc.dram_tensor("c", [H, BS], F32, kind="ExternalOutput")
    cb_d = nc.dram_tensor("cb", [H, BS], F32, kind="ExternalOutput")
    o_d = nc.dram_tensor("o", [H, BS], BF16, kind="ExternalOutput")
    dr_d = nc.dram_tensor("dr", [H, BS], BF16, kind="ExternalOutput")

    with tile.TileContext(nc) as tc:
        with (
            tc.tile_pool(name="wp", bufs=1) as wp,
            tc.tile_pool(name="xp", bufs=1) as xp,
            tc.tile_pool(name="cp", bufs=1) as cp,
            tc.tile_pool(name="gp", bufs=1) as gp,
            tc.tile_pool(name="dp", bufs=1) as dp,
            tc.tile_pool(name="op", bufs=2) as op_,
            tc.tile_pool(name="pp", bufs=4, space=bass.MemorySpace.PSUM) as pp,
        ):
            # --- input DMA issue order = arrival priority ---
            # weights arrive as one 1 MiB DMA per gate-group (fewer Sync
            # issues -> xh chunk issues are not starved during the ramp);
            # group 0 (d) is split so its first tile lands early.
            xh_sb = {}
            w_sb = [None] * 7
            w0a = None   # d-q0 weights: own tile -> own dep, earliest MM

            def load_xh(h, k):
                t = xp.tile([P, BN], BF16, tag=f"xh{h}_{k}")
                nc.sync.dma_start(t[:], xh_d[h, k])
                xh_sb[(h, k)] = t

            def load_w(grp):
                t = wp.tile([P, NQ * NK * P], BF16, tag=f"w{grp}")
                nc.sync.dma_start(t[:], w_d[grp])
                w_sb[grp] = t

            def w_ap(gt, k):
                grp, q = divmod(gt, NQ)
                if grp == 0:
                    if q == 0:
                        if k == 0:
                            return w0a1[:]
                        return w0a[:, (k - 1) * P:k * P]
                    base = (q - 1) * NK * P + k * P
                    return w_sb[0][:, base:base + P]
                base = q * NK * P + k * P
                return w_sb[grp][:, base:base + P]

            # PE clock warm-up: the tensor engine's DVFS ramps over ~3 us
            # of sustained use (first real matmuls run 608/427/379 ns vs
            # 216 steady, ~1.2 us lost). Run 8 dummy matmuls on a memset
            # tile while the PE would idle waiting for the first DMAs;
            # the clock survives short idle gaps (trace: full speed after
            # a 779 ns starve), so the real stream starts warm.
            # 7 dummies ~= 3.2 us at ramp clocks: ends just before the
            # first real matmul's dependency closure (~10.6 us) so the
            # warm-up never gates the stream start (PE executes in order)
            wsrc = xp.tile([P, 512], BF16, tag="warm")
            nc.any.memset(wsrc[:], 0)
            wacc = pp.tile([P, 512], F32, tag="acc", name="wacc")
            for i in range(7):
                nc.tensor.matmul(wacc[:], wsrc[:, :P], wsrc[:],
                                 start=(i == 0), stop=(i == 6))
            wout = gp.tile([P, 512], BF16, tag="warm_o")
            nc.scalar.activation(wout[:], wacc[:], AF.Sigmoid)

            load_xh(0, 0)
            # d-q0 k0 weights as their own 32 KiB DMA: the very first
            # matmul's dependency closure is xh00 + this tile instead of
            # the full 256 KiB d-q0 group (~0.6 us earlier stream start)
            w0a1 = wp.tile([P, P], BF16, tag="w0a1")
            nc.sync.dma_start(w0a1[:], w_d[0, :, :P])
            w0a = wp.tile([P, (NK - 1) * P], BF16, tag="w0a")
            nc.sync.dma_start(w0a[:], w_d[0, :, P:NK * P])
            load_xh(0, 1)
            w0b = wp.tile([P, 3 * NK * P], BF16, tag="w0b")
            nc.sync.dma_start(w0b[:], w_d[0, :, NK * P:])
            w_sb[0] = w0b
            load_xh(0, 2)
            load_xh(0, 3)
            for k in range(4, NK):
                load_xh(0, k)
            # bias after the xh chain: it is tiny and only needed at the
            # first PSUM drain (~1 us later), while every issue slot ahead
            # of xh07 delays the binding ramp path by ~0.6 us
            bb = cp.tile([P, NGT], F32, tag="bb")
            nc.sync.dma_start(bb[:], bb_d[:])
            load_w(1)
            load_w(2)
            for k in range(NK):
                load_xh(1, k)
            load_w(3)
            load_w(4)
            load_w(5)
            ct_sb = []
            for q in range(NQ):
                t = cp.tile([P, BS], BF16, tag=f"ct{q}")
                nc.sync.dma_start(t[:], ct_d[q * P:(q + 1) * P, :])
                ct_sb.append(t)
            load_w(6)

            # --- main loop: halves x gate-groups x quadrants ---
            for h in range(NH):
                col = slice(h * BN, (h + 1) * BN)
                # one [128, 4096] sigmoid(-wd) supertile per half: the Ln
                # is then a single ACT op, so the scheduler cannot
                # interleave it with sigmoid drains (one table switch
                # each way per half instead of per-quadrant)
                sdt = dp.tile([P, NQ * BN], BF16, tag="sd")
                gz = [None] * NQ
                gi = [None] * NQ
                gf = [None] * NQ
                gib = [None] * NQ
                gfb = [None] * NQ
                th = [None] * NQ

                def mm(gt):
                    # moving free dim caps at 512: two accumulation groups
                    # into the two banks of one [128, 1024] PSUM tile.
                    # k-outer/bank-inner reuses each stationary tile.
                    acc = pp.tile([P, BN], F32, tag="acc")
                    for k in range(NK):
                        for bh in range(2):
                            bsl = slice(bh * 512, (bh + 1) * 512)
                            nc.tensor.matmul(
                                acc[:, bsl], w_ap(gt, k),
                                xh_sb[(h, k)][:, bsl],
                                start=(k == 0), stop=(k == NK - 1),
                            )
                    return acc

                def mm_half(gt, bh):
                    # single-bank PSUM slot: drain of bank 0 depends only
                    # on its own 8 matmuls, overlapping bank 1's group
                    acc = pp.tile([P, 512], F32, tag="acc")
                    bsl = slice(bh * 512, (bh + 1) * 512)
                    for k in range(NK):
                        nc.tensor.matmul(
                            acc[:], w_ap(gt, k),
                            xh_sb[(h, k)][:, bsl],
                            start=(k == 0), stop=(k == NK - 1),
                        )
                    return acc

                def mm_first():
                    # d-q0 with PE pacing: the real stream starts warm and
                    # consumes xh k-chunks faster than the ~0.69 us/chunk
                    # DMA arrival rate, so k7 would idle the PE ~1.3 us -
                    # long enough to reset the p-state (costs ~1.9 us of
                    # 427 ns matmuls after). Weave 2 dep-free filler
                    # matmuls after each of the k4/k5/k6 pairs to keep the
                    # clock ramped through the xh07 wait.
                    acc = pp.tile([P, BN], F32, tag="acc")
                    wacc2 = pp.tile([P, 512], F32, tag="acc", name="wacc2")
                    for k in range(NK):
                        for bh in range(2):
                            bsl = slice(bh * 512, (bh + 1) * 512)
                            nc.tensor.matmul(
                                acc[:, bsl], w_ap(0, k),
                                xh_sb[(0, k)][:, bsl],
                                start=(k == 0), stop=(k == NK - 1),
                            )
                        if k in (4, 5, 6):
                            for _ in range(2):
                                nc.tensor.matmul(wacc2[:], wsrc[:, :P],
                                                 wsrc[:], start=True,
                                                 stop=True)
                    wout2 = gp.tile([P, 512], BF16, tag="warm_o")
                    nc.scalar.activation(wout2[:], wacc2[:], AF.Sigmoid)
                    return acc

                for grp in range(7):
                    for q in range(NQ):
                        gt = grp * NQ + q
                        rows = slice(q * P, (q + 1) * P)
                        last = (grp == GO and h == NH - 1 and q == NQ - 1)
                        if h == 0 and grp == 0 and q == 0:
                            acc = mm_first()
                        else:
                            acc = None if last else mm(gt)
                        bias = bb[:, gt:gt + 1]
                        if grp == GD:
                            # sigmoid(-(psum+b)) ; d-bias staged negated
                            nc.scalar.activation(
                                sdt[:, q * BN:(q + 1) * BN], acc[:],
                                AF.Sigmoid, bias=bias, scale=-1.0)
                        elif grp == GZ:
                            g = gp.tile([P, BN], BF16, tag=f"z{q}")
                            nc.scalar.activation(g[:], acc[:], AF.Tanh,
                                                 bias=bias)
                            gz[q] = g
                        elif last:
                            # final tile: per-bank PSUM slots + drains so
                            # the tail after the very last matmul is one
                            # 512-col sigmoid + mul + store
                            g = gp.tile([P, BN], BF16, tag=f"o{q}")
                            hh = op_.tile([P, BN], BF16, tag="hh")
                            for b2 in range(2):
                                acch = mm_half(gt, b2)
                                ssl = slice(b2 * 512, (b2 + 1) * 512)
                                csl = slice(h * BN + b2 * 512,
                                            h * BN + (b2 + 1) * 512)
                                nc.scalar.activation(g[:, ssl], acch[:],
                                                     AF.Sigmoid, bias=bias)
                                nc.sync.dma_start(o_d[rows, csl], g[:, ssl])
                                nc.vector.tensor_mul(hh[:, ssl], g[:, ssl],
                                                     th[q][:, ssl])
                                nc.sync.dma_start(h_d[rows, csl], hh[:, ssl])
                        else:
                            tagn = ("", "", "i", "f", "ib", "fb", "o")[grp]
                            g = gp.tile([P, BN], BF16, tag=f"{tagn}{q}")
                            nc.scalar.activation(g[:], acc[:], AF.Sigmoid,
                                                 bias=bias)
                            if grp == GI:
                                gi[q] = g
                            elif grp == GF:
                                gf[q] = g
                            elif grp == GIB:
                                gib[q] = g
                            elif grp == GFB:
                                gfb[q] = g
                                # chain part A per quadrant, right after
                                # its fb sigmoid: spreads the tanh(c) ACT
                                # ops so PSUM drains are never delayed
                                ctq = ct_sb[q][:, col]
                                c = op_.tile([P, BN], F32, tag="c")
                                tmp = op_.tile([P, BN], F32, tag="tmp")
                                cb = op_.tile([P, BN], F32, tag="cb")
                                # all 4 live until chain B: bufs=4
                                t_ = op_.tile([P, BN], BF16, tag="th",
                                              bufs=4)
                                nc.vector.tensor_mul(c[:], gf[q][:], ctq)
                                nc.vector.tensor_mul(tmp[:], gi[q][:], gz[q][:])
                                nc.vector.tensor_add(c[:], c[:], tmp[:])
                                nc.sync.dma_start(c_d[rows, col], c[:])
                                nc.scalar.activation(t_[:], c[:], AF.Tanh)
                                th[q] = t_
                                nc.vector.tensor_mul(cb[:], gfb[q][:], ctq)
                                nc.vector.tensor_mul(tmp[:], gib[q][:], gz[q][:])
                                nc.vector.tensor_add(cb[:], cb[:], tmp[:])
                                nc.sync.dma_start(cb_d[rows, col], cb[:])
                            else:  # GO: chain part B
                                nc.sync.dma_start(o_d[rows, col], g[:])
                                hh = op_.tile([P, BN], BF16, tag="hh")
                                nc.vector.tensor_mul(hh[:], g[:], th[q][:])
                                nc.sync.dma_start(h_d[rows, col], hh[:])
                    if grp == GD:
                        # softplus for this half: -ln(sigmoid(-wd)),
                        # single Ln + negate over the supertile
                        nc.scalar.activation(sdt[:], sdt[:], AF.Ln)
                        nc.vector.tensor_scalar_mul(sdt[:], sdt[:], -1.0)
                        for q in range(NQ):
                            rows = slice(q * P, (q + 1) * P)
                            nc.sync.dma_start(dr_d[rows, col],
                                              sdt[:, q * BN:(q + 1) * BN])

    nc.compile()
    return nc


def kernel(x, ht, ct, Wx, bx, Wh, bh):
    global _nc_cache, LAST_RESULTS
    if _nc_cache is None:
        _nc_cache = _build()
    nc = _nc_cache

    x = np.ascontiguousarray(x, dtype=np.float32)
    ht = np.ascontiguousarray(ht, dtype=np.float32)
    ct = np.ascontiguousarray(ct, dtype=np.float32)

    # weights: [K2, G] in exec gate order, tiled [28][128][8*128]
    WxT = np.asarray(Wx, dtype=np.float32).T   # [512, 3584]
    WhT = np.asarray(Wh, dtype=np.float32).T
    bsum = np.asarray(bx, dtype=np.float32) + np.asarray(bh, dtype=np.float32)
    w2 = np.empty((K2, G), dtype=NPBF16)
    bbp = np.empty(G, dtype=np.float32)
    for n, old in enumerate(SRC):
        dsl = slice(n * H, (n + 1) * H)
        ssl = slice(old * H, (old + 1) * H)
        w2[:I, dsl] = WxT[:, ssl].astype(NPBF16)
        w2[I:, dsl] = WhT[:, ssl].astype(NPBF16)
        bbp[dsl] = bsum[ssl]
    bbp[0:H] = -bbp[0:H]           # d-gate bias negated (scale=-1 trick)
    # w_stage[grp, p, q*1024 + k*128 + g] = w2[k*128+p, (grp*4+q)*128+g]
    w_stage = np.ascontiguousarray(
        w2.reshape(NK, P, 7, NQ, P).transpose(2, 1, 3, 0, 4)
        .reshape(7, P, NQ * NK * P)
    )
    bbT = np.ascontiguousarray(bbp.reshape(NGT, P).T)   # [128, 28]

    in_maps = []
    for cidx in range(NCORES):
        sl = slice(cidx * BS, (cidx + 1) * BS)
        xh_full = np.empty((K2, BS), dtype=NPBF16)
        xh_full[:I, :] = x[sl].T.astype(NPBF16)
        xh_full[I:, :] = ht[sl].T.astype(NPBF16)
        # [2, 8, 128, 1024] halves-major
        xh_stage = np.ascontiguousarray(
            xh_full.reshape(NK, P, NH, BN).transpose(2, 0, 1, 3)
        )
        ctT = np.ascontiguousarray(ct[sl].T.astype(NPBF16))
        in_maps.append({
            "xh": xh_stage,
            "w": w_stage,
            "ct": ctT,
            "bb": bbT,
        })

    res = run_bass_kernel_spmd(nc, in_maps, core_ids=list(range(NCORES)),
                               trace=TRACE)
    LAST_RESULTS = res

    outs = {}
    for name in ("h", "c", "cb", "o", "dr"):
        full = np.concatenate(
            [res.results[cidx][name] for cidx in range(NCORES)], axis=1
        )
        outs[name] = np.ascontiguousarray(full.T.astype(np.float32))
    return outs["h"], outs["c"], outs["cb"], outs["o"], outs["dr"]


# revision 39
# speedup vs baseline: 1.1946x; 1.1946x over previous
"""CTLSTM cell fused kernel for 8 Trainium2 NeuronCores.

Strategy (data-parallel over batch, TRANSPOSED compute):
  - B=16384 rows sharded 2048/core; weights replicated.
  - Compute g.T: gates on SBUF partitions, batch on the free dim.
    Stationary operand = weight tile [K=128, 128 gates]; moving operand =
    xh [K=128, 1024 batch] bf16 (max bf16 moving free dim). Each PSUM tile
    is one gate-tile x batch-half: [128, 1024] fp32 (2 banks).
  - With gates on partitions the bias is per-partition: the ACT engine
    drains PSUM directly with out = act(psum*scale + bias[p]) in ONE op -
    no DVE bias-add drain at all. DVE only runs the elementwise chain.
  - Gate-group exec order [d, z, i, f, ib, fb, o]:
      * d first: softplus(wd) = -ln(sigmoid(-wd)). The Ln burst (one act
        table switch to natural_log and back) slots in right after the
        4 d-tiles of each half, mid-stream where ACT has slack.
      * o last: the tail after the final matmul is just sigmoid(o) +
        h = o*tanh(c) + store.
  - DMA priority: xh half-0 chunks first, weights as one ~1 MiB DMA per
    gate-group (d-q0 split into its own tile for the earliest matmul);
    few Sync issues keep the ramp short and weight arrival stays ahead
    of the PE for the rest of the kernel. (A further 32 KiB k0-split of
    w0a was reverted: one run produced inf in decay_rate - the output
    fed by that weight tile - and could not be re-validated in budget.)
  - Outputs h/o/dr stored bf16 (halved write traffic), c/cb fp32.
    Host transposes back and upcasts.

  - PE CLOCK WARM-UP + PACING: the tensor engine's DVFS ramps over ~3 us
    of sustained use and resets on idle gaps over ~1 us. Eight dummy
    matmuls on a memset tile run while the first DMAs stream (PE would
    idle anyway), so the real stream starts at full 2.4 GHz; two filler
    matmuls after each of the d-q0 k4/k5/k6 pairs bridge the xh07
    arrival wait so the clock never drops mid-ramp. Trace-verified:
    real-stream busy 194.8 -> 193.4 us, window 196.7 -> 195.3 us.

Measured: 213.0-216.5 us HW exec with warm-up+pacing (median 214.5,
best 213.0) vs 214.5-218.0 us without (median 215.7) - identical-code
run noise is +-1.8 us, so compare medians/trace metrics, not single
runs. Naive baseline 302.5 us. bf16 matmul roofline 191.1 us; real PE
busy 193.4 us (excess = hw-decode 2.2 ns/instr + 17 periodic ~216 ns
instruction-buffer refills every ~100 PE instructions). Head ~12 us =
7.2 fixed NEFF preamble + 3 MB ramp-critical DMA at ~370 GB/s + 0.9 us
DMA sem-prop. Tail ~6 us = sigmoid+mul+store chain (issue 625 + DGE
650 + sem-prop 900 fixed, compute-gated) + ~2.8 us teardown.
Theoretical floor with this structure ~210 us.

Tried and rejected (do not retry):
  - fp8 e4m3 DoubleRow matmul (1.44x PE measured): numerically FAILS the
    2e-2 gate - e4m3 both-operands gives 0.081 worst scale-rel, w-split
    0.062; the 3-pass hi/lo split passes (0.0027) but costs 3/1.44 = 2.1x
    bf16 time. bf16 is the floor for this accuracy.
  - 1024-col moving matmul (halve instr count): walrus ISA check rejects
    it (s3d3_mm_num_elements); 512 is the hw max despite bass not
    asserting.
  - k-wave ramp (d-group k-outer over 4 open PSUM groups + k-major
    staged xh/w0) with waves on sync: scheduler re-plans globally;
    measured 215.2-226.6 us across 4 variants - never beat this order.
  - weight DMAs on the Act HWDGE queue: Q_X starts ~2.9 us after issue
    and steals ~50% bandwidth from the sync ramp when active; anything
    the first 15 us needs must be on sync, in this exact order.
  - final o-store on Act queue: the DIRECT2D issue on the Scalar
    sequencer delays its own final sigmoids; +1.8 us.
  - walrus --enable-ldw-opt=true (merge the 448 redundant consecutive
    LDWEIGHTS from the bank-pair weight reuse): crashes codegen in
    visitInstLdweights (CoreV3GenImpl.cpp:694) - disabled for a reason.
Run-to-run noise on shared hw is larger than it looks: identical code
measured 214.5-218.0 us across 5 runs in one session.
"""

import numpy as np
import ml_dtypes

import concourse.bacc as bacc
import concourse.bass as bass
import concourse.mybir as mybir
import concourse.tile as tile
from concourse.bass_utils import run_bass_kernel_spmd

NCORES = 8
B = 16384
I = 512
H = 512
NG = 7
G = NG * H          # 3584
K2 = I + H          # 1024
P = 128
BS = B // NCORES    # 2048 batch cols per core
NH = 2              # batch halves of 1024
BN = BS // NH       # 1024
NQ = H // P         # 4 hidden quadrants (128 gate rows each)
NK = K2 // P        # 8 contraction chunks
NGT = G // P        # 28 gate tiles

BF16 = mybir.dt.bfloat16
F32 = mybir.dt.float32
AF = mybir.ActivationFunctionType
NPBF16 = ml_dtypes.bfloat16

# gate-group exec order: d, z, i, f, ib, fb, o
# reference row order is   i, f, z, o, d, ib, fb
SRC = [4, 2, 0, 1, 5, 6, 3]
GD, GZ, GI, GF, GIB, GFB, GO = range(7)

TRACE = False
LAST_RESULTS = None

_nc_cache = None


def _build():
    nc = bacc.Bacc("TRN2", target_bir_lowering=False, debug=False)

    xh_d = nc.dram_tensor("xh", [NH, NK, P, BN], BF16, kind="ExternalInput")
    w_d = nc.dram_tensor("w", [7, P, NQ * NK * P], BF16, kind="ExternalInput")
    ct_d = nc.dram_tensor("ct", [H, BS], BF16, kind="ExternalInput")
    bb_d = nc.dram_tensor("bb", [P, NGT], F32, kind="ExternalInput")

    h_d = nc.dram_tensor("h", [H, BS], BF16, kind="ExternalOutput")
    c_d = nc.dram_tensor("c", [H, BS], F32, kind="ExternalOutput")
    cb_d = nc.dram_tensor("cb", [H, BS], F32, kind="ExternalOutput")
    o_d = nc.dram_tensor("o", [H, BS], BF16, kind="ExternalOutput")
    dr_d = nc.dram_tensor("dr", [H, BS], BF16, kind="ExternalOutput")

    with tile.TileContext(nc) as tc:
        with (
            tc.tile_pool(name="wp", bufs=1) as wp,
            tc.tile_pool(name="xp", bufs=1) as xp,
            tc.tile_pool(name="cp", bufs=1) as cp,
            tc.tile_pool(name="gp", bufs=1) as gp,
            tc.tile_pool(name="dp", bufs=1) as dp,
            tc.tile_pool(name="op", bufs=2) as op_,
            tc.tile_pool(name="pp", bufs=4, space=bass.MemorySpace.PSUM) as pp,
        ):
            # --- input DMA issue order = arrival priority ---
            # weights arrive as one 1 MiB DMA per gate-group (fewer Sync
            # issues -> xh chunk issues are not starved during the ramp);
            # group 0 (d) is split so its first tile lands early.
            xh_sb = {}
            w_sb = [None] * 7
            w0a = None   # d-q0 weights: own tile -> own dep, earliest MM

            def load_xh(h, k):
                t = xp.tile([P, BN], BF16, tag=f"xh{h}_{k}")
                nc.sync.dma_start(t[:], xh_d[h, k])
                xh_sb[(h, k)] = t

            def load_w(grp):
                t = wp.tile([P, NQ * NK * P], BF16, tag=f"w{grp}")
                nc.sync.dma_start(t[:], w_d[grp])
                w_sb[grp] = t

            def w_ap(gt, k):
                grp, q = divmod(gt, NQ)
                if grp == 0:
                    if q == 0:
                        return w0a[:, k * P:(k + 1) * P]
                    base = (q - 1) * NK * P + k * P
                    return w_sb[0][:, base:base + P]
                base = q * NK * P + k * P
                return w_sb[grp][:, base:base + P]

            # PE clock warm-up: the tensor engine's DVFS ramps over ~3 us
            # of sustained use (first real matmuls run 608/427/379 ns vs
            # 216 steady, ~1.2 us lost). Run 8 dummy matmuls on a memset
            # tile while the PE would idle waiting for the first DMAs;
            # the clock survives short idle gaps (trace: full speed after
            # a 779 ns starve), so the real stream starts warm.
            # 6 dummies ~= 2.8 us at ramp clocks: ends just before the
            # first real matmul's dependency closure (~10.6 us) so the
            # warm-up never gates the stream start (PE executes in order)
            wsrc = xp.tile([P, 512], BF16, tag="warm")
            nc.any.memset(wsrc[:], 0)
            wacc = pp.tile([P, 512], F32, tag="acc", name="wacc")
            for i in range(6):
                nc.tensor.matmul(wacc[:], wsrc[:, :P], wsrc[:],
                                 start=(i == 0), stop=(i == 5))
            wout = gp.tile([P, 512], BF16, tag="warm_o")
            nc.scalar.activation(wout[:], wacc[:], AF.Sigmoid)

            load_xh(0, 0)
            w0a = wp.tile([P, NK * P], BF16, tag="w0a")
            nc.sync.dma_start(w0a[:], w_d[0, :, :NK * P])
            load_xh(0, 1)
            w0b = wp.tile([P, 3 * NK * P], BF16, tag="w0b")
            nc.sync.dma_start(w0b[:], w_d[0, :, NK * P:])
            w_sb[0] = w0b
            load_xh(0, 2)
            load_xh(0, 3)
            for k in range(4, NK):
                load_xh(0, k)
            # bias after the xh chain: it is tiny and only needed at the
            # first PSUM drain (~1 us later), while every issue slot ahead
            # of xh07 delays the binding ramp path by ~0.6 us
            bb = cp.tile([P, NGT], F32, tag="bb")
            nc.sync.dma_start(bb[:], bb_d[:])
            load_w(1)
            load_w(2)
            for k in range(NK):
                load_xh(1, k)
            load_w(3)
            load_w(4)
            load_w(5)
            ct_sb = []
            for q in range(NQ):
                t = cp.tile([P, BS], BF16, tag=f"ct{q}")
                nc.sync.dma_start(t[:], ct_d[q * P:(q + 1) * P, :])
                ct_sb.append(t)
            load_w(6)

            # --- main loop: halves x gate-groups x quadrants ---
            for h in range(NH):
                col = slice(h * BN, (h + 1) * BN)
                # one [128, 4096] sigmoid(-wd) supertile per half: the Ln
                # is then a single ACT op, so the scheduler cannot
                # interleave it with sigmoid drains (one table switch
                # each way per half instead of per-quadrant)
                sdt = dp.tile([P, NQ * BN], BF16, tag="sd")
                gz = [None] * NQ
                gi = [None] * NQ
                gf = [None] * NQ
                gib = [None] * NQ
                gfb = [None] * NQ
                th = [None] * NQ

                def mm(gt):
                    # moving free dim caps at 512: two accumulation groups
                    # into the two banks of one [128, 1024] PSUM tile.
                    # k-outer/bank-inner reuses each stationary tile.
                    acc = pp.tile([P, BN], F32, tag="acc")
                    for k in range(NK):
                        for bh in range(2):
                            bsl = slice(bh * 512, (bh + 1) * 512)
                            nc.tensor.matmul(
                                acc[:, bsl], w_ap(gt, k),
                                xh_sb[(h, k)][:, bsl],
                                start=(k == 0), stop=(k == NK - 1),
                            )
                    return acc

                def mm_half(gt, bh):
                    # single-bank PSUM slot: drain of bank 0 depends only
                    # on its own 8 matmuls, overlapping bank 1's group
                    acc = pp.tile([P, 512], F32, tag="acc")
                    bsl = slice(bh * 512, (bh + 1) * 512)
                    for k in range(NK):
                        nc.tensor.matmul(
                            acc[:], w_ap(gt, k),
                            xh_sb[(h, k)][:, bsl],
                            start=(k == 0), stop=(k == NK - 1),
                        )
                    return acc

                def mm_first():
                    # d-q0 with PE pacing: the real stream starts warm and
                    # consumes xh k-chunks faster than the ~0.69 us/chunk
                    # DMA arrival rate, so k7 would idle the PE ~1.3 us -
                    # long enough to reset the p-state (costs ~1.9 us of
                    # 427 ns matmuls after). Weave 2 dep-free filler
                    # matmuls after each of the k4/k5/k6 pairs to keep the
                    # clock ramped through the xh07 wait.
                    acc = pp.tile([P, BN], F32, tag="acc")
                    wacc2 = pp.tile([P, 512], F32, tag="acc", name="wacc2")
                    for k in range(NK):
                        for bh in range(2):
                            bsl = slice(bh * 512, (bh + 1) * 512)
                            nc.tensor.matmul(
                                acc[:, bsl], w_ap(0, k),
                                xh_sb[(0, k)][:, bsl],
                                start=(k == 0), stop=(k == NK - 1),
                            )
                        if k in (4, 5, 6):
                            for _ in range(2):
                                nc.tensor.matmul(wacc2[:], wsrc[:, :P],
                                                 wsrc[:], start=True,
                                                 stop=True)
                    wout2 = gp.tile([P, 512], BF16, tag="warm_o")
                    nc.scalar.activation(wout2[:], wacc2[:], AF.Sigmoid)
                    return acc

                for grp in range(7):
                    for q in range(NQ):
                        gt = grp * NQ + q
                        rows = slice(q * P, (q + 1) * P)
                        last = (grp == GO and h == NH - 1 and q == NQ - 1)
                        if h == 0 and grp == 0 and q == 0:
                            acc = mm_first()
                        else:
                            acc = None if last else mm(gt)
                        bias = bb[:, gt:gt + 1]
                        if grp == GD:
                            # sigmoid(-(psum+b)) ; d-bias staged negated
                            nc.scalar.activation(
                                sdt[:, q * BN:(q + 1) * BN], acc[:],
                                AF.Sigmoid, bias=bias, scale=-1.0)
                        elif grp == GZ:
                            g = gp.tile([P, BN], BF16, tag=f"z{q}")
                            nc.scalar.activation(g[:], acc[:], AF.Tanh,
                                                 bias=bias)
                            gz[q] = g
                        elif last:
                            # final tile: per-bank PSUM slots + drains so
                            # the tail after the very last matmul is one
                            # 512-col sigmoid + mul + store
                            g = gp.tile([P, BN], BF16, tag=f"o{q}")
                            hh = op_.tile([P, BN], BF16, tag="hh")
                            for b2 in range(2):
                                acch = mm_half(gt, b2)
                                ssl = slice(b2 * 512, (b2 + 1) * 512)
                                csl = slice(h * BN + b2 * 512,
                                            h * BN + (b2 + 1) * 512)
                                nc.scalar.activation(g[:, ssl], acch[:],
                                                     AF.Sigmoid, bias=bias)
                                nc.sync.dma_start(o_d[rows, csl], g[:, ssl])
                                nc.vector.tensor_mul(hh[:, ssl], g[:, ssl],
                                                     th[q][:, ssl])
                                nc.sync.dma_start(h_d[rows, csl], hh[:, ssl])
                        else:
                            tagn = ("", "", "i", "f", "ib", "fb", "o")[grp]
                            g = gp.tile([P, BN], BF16, tag=f"{tagn}{q}")
                            nc.scalar.activation(g[:], acc[:], AF.Sigmoid,
                                                 bias=bias)
                            if grp == GI:
                                gi[q] = g
                            elif grp == GF:
                                gf[q] = g
                            elif grp == GIB:
                                gib[q] = g
                            elif grp == GFB:
                                gfb[q] = g
                                # chain part A per quadrant, right after
                                # its fb sigmoid: spreads the tanh(c) ACT
                                # ops so PSUM drains are never delayed
                                ctq = ct_sb[q][:, col]
                                c = op_.tile([P, BN], F32, tag="c")
                                tmp = op_.tile([P, BN], F32, tag="tmp")
                                cb = op_.tile([P, BN], F32, tag="cb")
                                # all 4 live until chain B: bufs=4
                                t_ = op_.tile([P, BN], BF16, tag="th",
                                              bufs=4)
                                nc.vector.tensor_mul(c[:], gf[q][:], ctq)
                                nc.vector.tensor_mul(tmp[:], gi[q][:], gz[q][:])
                                nc.vector.tensor_add(c[:], c[:], tmp[:])
                                nc.sync.dma_start(c_d[rows, col], c[:])
                                nc.scalar.activation(t_[:], c[:], AF.Tanh)
                                th[q] = t_
                                nc.vector.tensor_mul(cb[:], gfb[q][:], ctq)
                                nc.vector.tensor_mul(tmp[:], gib[q][:], gz[q][:])
                                nc.vector.tensor_add(cb[:], cb[:], tmp[:])
                                nc.sync.dma_start(cb_d[rows, col], cb[:])
                            else:  # GO: chain part B
                                nc.sync.dma_start(o_d[rows, col], g[:])
                                hh = op_.tile([P, BN], BF16, tag="hh")
                                nc.vector.tensor_mul(hh[:], g[:], th[q][:])
                                nc.sync.dma_start(h_d[rows, col], hh[:])
                    if grp == GD:
                        # softplus for this half: -ln(sigmoid(-wd)),
                        # single Ln + negate over the supertile
                        nc.scalar.activation(sdt[:], sdt[:], AF.Ln)
                        nc.vector.tensor_scalar_mul(sdt[:], sdt[:], -1.0)
                        for q in range(NQ):
                            rows = slice(q * P, (q + 1) * P)
                            nc.sync.dma_start(dr_d[rows, col],
                                              sdt[:, q * BN:(q + 1) * BN])

    nc.compile()
    return nc


def kernel(x, ht, ct, Wx, bx, Wh, bh):
    global _nc_cache, LAST_RESULTS
    if _nc_cache is None:
        _nc_cache = _build()
    nc = _nc_cache

    x = np.ascontiguousarray(x, dtype=np.float32)
    ht = np.ascontiguousarray(ht, dtype=np.float32)
    ct = np.ascontiguousarray(ct, dtype=np.float32)

    # weights: [K2, G] in exec gate order, tiled [28][128][8*128]
    WxT = np.asarray(Wx, dtype=np.float32).T   # [512, 3584]
    WhT = np.asarray(Wh, dtype=np.float32).T
    bsum = np.asarray(bx, dtype=np.float32) + np.asarray(bh, dtype=np.float32)
    w2 = np.empty((K2, G), dtype=NPBF16)
    bbp = np.empty(G, dtype=np.float32)
    for n, old in enumerate(SRC):
        dsl = slice(n * H, (n + 1) * H)
        ssl = slice(old * H, (old + 1) * H)
        w2[:I, dsl] = WxT[:, ssl].astype(NPBF16)
        w2[I:, dsl] = WhT[:, ssl].astype(NPBF16)
        bbp[dsl] = bsum[ssl]
    bbp[0:H] = -bbp[0:H]           # d-gate bias negated (scale=-1 trick)
    # w_stage[grp, p, q*1024 + k*128 + g] = w2[k*128+p, (grp*4+q)*128+g]
    w_stage = np.ascontiguousarray(
        w2.reshape(NK, P, 7, NQ, P).transpose(2, 1, 3, 0, 4)
        .reshape(7, P, NQ * NK * P)
    )
    bbT = np.ascontiguousarray(bbp.reshape(NGT, P).T)   # [128, 28]

    in_maps = []
    for cidx in range(NCORES):
        sl = slice(cidx * BS, (cidx + 1) * BS)
        xh_full = np.empty((K2, BS), dtype=NPBF16)
        xh_full[:I, :] = x[sl].T.astype(NPBF16)
        xh_full[I:, :] = ht[sl].T.astype(NPBF16)
        # [2, 8, 128, 1024] halves-major
        xh_stage = np.ascontiguousarray(
            xh_full.reshape(NK, P, NH, BN).transpose(2, 0, 1, 3)
        )
        ctT = np.ascontiguousarray(ct[sl].T.astype(NPBF16))
        in_maps.append({
            "xh": xh_stage,
            "w": w_stage,
            "ct": ctT,
            "bb": bbT,
        })

    res = run_bass_kernel_spmd(nc, in_maps, core_ids=list(range(NCORES)),
                               trace=TRACE)
    LAST_RESULTS = res

    outs = {}
    for name in ("h", "c", "cb", "o", "dr"):
        full = np.concatenate(
            [res.results[cidx][name] for cidx in range(NCORES)], axis=1
        )
        outs[name] = np.ascontiguousarray(full.T.astype(np.float32))
    return outs["h"], outs["c"], outs["cb"], outs["o"], outs["dr"]
